# revision 1
# baseline (speedup 1.0000x reference)
"""Trainium2 Bass kernel for BiDAF-style bidirectional attention.

Reference computation (per batch element n; M=1 folded away):
    s[i,j]  = h[i].w_h + u[j].w_u + (h[i]*u[j]).w_hu + b      [JX, JQ]
    a_u     = softmax_j(s);     u_a[i] = sum_j a_u[i,j] u[j]   (c2q)
    a_h     = softmax_i(max_j s);  h_a = sum_i a_h[i] h[i]     (q2c)
    out     = concat(h, u_a, h*u_a, h*h_a)                     [JX, 4D]

Sharding: data-parallel over batch N=8, one NeuronCore per batch element.
alpha_b drops out entirely (both softmaxes are shift-invariant).

Key algebra vs the straightforward mapping:
  - w_h folds into the score weights: with uw'[j,d] = u[j,d]*w_hu[d]+w_h[d],
    sT[j,i] = sum_d uw'[j,d] h[i,d] = (h.w_h)[i] + ((h*w_hu).u)[i,j], so
    ET = exp(sT + uwu[j]) = exp(s - b) exactly and the whole h.w_h pass
    (PSUM row accumulators + evictions) disappears.
  - alpha_w is loaded once as a [1,1536] row (one descriptor) and partition-
    broadcast ON-CHIP via three K=1 matmuls into PSUM which the DVE reads
    directly.  (A [P,3D] broadcast DMA is 128 x 6KB descriptors on one
    queue: ~20+ us, and it sat on the critical path to the scores.)
  - f32r-at-source dtype scheme: tensors feeding f32r matmuls are TYPED
    f32r at their producer (DMA loads via source bitcast, DVE/ACT writers
    emit f32r) so the walrus FP32r verifier passes with no conversion
    copies; f32 consumers read the same bits via .bitcast(f32).  This
    deletes ~5 us of ScalarE h_r/u_r/ident copies the old version needed.
  - all PE transposes use the f32r identity as the moving operand
    (1.5 cycles/row instead of 2.0 for f32).

DMA structure (kept deliberately coarse: each DMA instruction costs ~0.6us
sequencer issue + ~0.9us completion semaphore, and all queues share one
~300GB/s bus, so few big transfers beat many small ones):
  - h: 8 whole-tile loads + u + aw on sync;
  - h passthrough (col 0) per tile on gpsimd, tiles >= 4 gated on block-0's
    exp so they land in the mid-kernel DMA window;
  - stg (cols 1-2, one [P,2D] 4KB-row DMA per tile) + o4 (col 3) on sync.
Per-core dataflow: PE warmup opens the clock gate under the h loads; hT via
32 PE transposes (evictions split Scalar/DVE); scores per 512-block (4
K=128 matmuls, ScalarE Exp evict with bias=uwu); ET re-transposed for DVE
max/sum reduces; q2c chain right after block-1 reduce so o4 = h*h_a writes
spread out instead of cramming at the end.
"""

import numpy as np

N_B, M_B, JX, JQ, D = 8, 1, 1024, 128, 512
P = 128
NT = JX // P   # 8 i-tiles
KC = D // P    # 4 d-chunks
IB = 512       # i-block width for score matmuls
NB = JX // IB  # 2 blocks
TPB = NT // NB  # tiles per block

_CACHE = {}


def _build_program():
    from contextlib import ExitStack

    import concourse.bass as bass
    import concourse.tile as tile
    from concourse import bacc, mybir
    from concourse.masks import make_identity
    from concourse.tile_rust import add_dep_helper

    f32 = mybir.dt.float32
    f32r = mybir.dt.float32r
    EXP = mybir.ActivationFunctionType.Exp
    AX = mybir.AxisListType.X
    ds = bass.ds

    nc = bacc.Bacc("TRN2", target_bir_lowering=False, debug=False, num_devices=8)
    h_d = nc.dram_tensor("h", [JX, D], f32, kind="ExternalInput").ap()
    u_d = nc.dram_tensor("u", [JQ, D], f32, kind="ExternalInput").ap()
    aw_d = nc.dram_tensor("alpha_w", [3 * D], f32, kind="ExternalInput").ap()
    out_d = nc.dram_tensor("out", [JX, 4 * D], f32, kind="ExternalOutput").ap()

    with tile.TileContext(nc) as tc, ExitStack() as ctx:
        consts = ctx.enter_context(tc.tile_pool(name="consts", bufs=1))
        stage = ctx.enter_context(tc.tile_pool(name="stage", bufs=6))
        # PSUM budget (8 banks): tp=2, s0=2, ua=2, acc=1, hap=1
        ps = ctx.enter_context(tc.tile_pool(name="ps", bufs=2, space="PSUM"))

        # ---- PE warmup: f32r matmuls depending only on DVE ops, emitted
        # first so the HAM clock-gate opens (1.2 -> 2.4 GHz) while the h
        # DMAs stream in.
        warm_f = consts.tile([P, D], f32)
        nc.vector.memset(warm_f[:], 0.25)
        warm = consts.tile([P, D], f32r)
        nc.vector.tensor_copy(warm[:], warm_f[:])
        wp = ps.tile([P, D], f32, tag="acc", bufs=1)
        for w in range(4):
            nc.tensor.matmul(
                wp[:], warm[:, ds(0, P)], warm[:], start=True, stop=True,
            )

        # ---- constants / prep ----
        ident_f = consts.tile([P, P], f32)
        make_identity(nc, ident_f[:])
        ident = consts.tile([P, P], f32r)
        nc.vector.tensor_copy(ident[:], ident_f[:])
        ones_row_f = consts.tile([1, P], f32)
        nc.vector.memset(ones_row_f[:], 1.0)
        ones_row = consts.tile([1, P], f32r)
        nc.scalar.copy(ones_row[:], ones_row_f[:])
        ones_col = consts.tile([P, 1], f32)
        nc.vector.memset(ones_col[:], 1.0)

        u_sb = consts.tile([JQ, D], f32r)
        nc.sync.dma_start(u_sb[:], u_d[:].bitcast(f32r))
        u_f = u_sb[:].bitcast(f32)
        aw_sb = consts.tile([1, 3 * D], f32r)
        nc.sync.dma_start(aw_sb[:], aw_d.rearrange("(o d) -> o d", o=1).bitcast(f32r))

        # alpha_w partition-broadcast on-chip: K=1 matmuls into s0 PSUM,
        # read directly by the DVE.  whu first (it gates the uw' multiply).
        def wcast(c):
            wt = ps.tile([P, D], f32, tag="s0")
            nc.tensor.matmul(
                wt[:], ones_row[:], aw_sb[:, ds(c * D, D)], start=True, stop=True
            )
            return wt

        whu_p = wcast(2)
        wh_p = wcast(0)

        # uw[j,d] = u[j,d]*w_hu[d] + w_h[d];  uwu[j] = sum_d u[j,d]*w_u[d]
        uw = consts.tile([JQ, D], f32r)
        uw0 = consts.tile([JQ, D], f32)
        nc.vector.tensor_mul(uw0[:], u_f, whu_p[:])
        nc.vector.tensor_add(uw[:], uw0[:], wh_p[:])

        # ---- load h; passthrough out1; build hT ----
        h_all = consts.tile([P, NT * D], f32r)    # tile t: h[t*128+p, d]
        h_f = h_all[:].bitcast(f32)
        hT_all = consts.tile([P, KC * JX], f32r)  # chunk k: hT[k*128+p, i]
        hT3 = hT_all[:].rearrange("p (k x) -> p k x", k=KC)
        hout_late = []
        for t in range(NT):
            nc.sync.dma_start(
                h_all[:, ds(t * D, D)], h_d[ds(t * P, P), :].bitcast(f32r)
            )
            # out1 = h passthrough (GpSimd DMA queue; Sync stays free).
            # Tiles 0-3 flow as soon as their load lands - they bridge the
            # bus between the load phase and the write phase; tiles 4-7 are
            # gated on block-0's exp so they don't delay the block-1 loads.
            ho = nc.gpsimd.dma_start(
                out_d[ds(t * P, P), ds(0, D)].bitcast(f32r), h_all[:, ds(t * D, D)]
            )
            if t >= NT // 2:
                hout_late.append(ho)

        def transpose_tile(t):
            pt = ps.tile([P, KC * P], f32r, tag="tp")
            for k in range(KC):
                nc.tensor.transpose(
                    pt[:, ds(k * P, P)], h_all[:, ds(t * D + k * P, P)], ident[:]
                )
            ev = nc.scalar.copy if t % 2 == 0 else nc.vector.tensor_copy
            ev(hT3[:, :, ds(t * P, P)], pt[:].rearrange("p (k x) -> p k x", k=KC))

        # ---- PE spine in expected-readiness order.  Block 0's transposes
        # run straight after warmup (tile 0 lands early); the uwT transposes
        # slot in behind them, hiding the DVE uw' chain; block-0 c2q writes
        # stream while block-1 tiles are still loading.
        ET = consts.tile([JQ, JX], f32r)          # exp(sT + uwu[j]) = exp(s - b)
        m_exp = consts.tile([P, NT], f32r)        # per i-tile: max_j ET
        z_rec = consts.tile([P, NT], f32)         # per i-tile: 1/sum_j ET
        hap = ps.tile([1, D], f32, tag="hap", bufs=1)

        for t in range(TPB):
            transpose_tile(t)

        wu_p = wcast(1)
        uwtmp = consts.tile([JQ, D], f32)
        uwu = consts.tile([JQ, 1], f32)
        nc.vector.scalar_tensor_tensor(
            uwtmp[:], u_f, 1.0, wu_p[:],
            op0=mybir.AluOpType.mult, op1=mybir.AluOpType.mult, accum_out=uwu[:],
        )
        # uwT[d_chunk][j]: 4 transposes into one PSUM bank, one batched evict
        uwT = consts.tile([P, KC * JQ], f32r)
        ptw = ps.tile([P, KC * P], f32r, tag="tp")
        for k in range(KC):
            nc.tensor.transpose(ptw[:, ds(k * P, P)], uw[:, ds(k * P, P)], ident[:])
        nc.scalar.copy(uwT[:], ptw[:])

        def block_scores(b):
            sp = ps.tile([JQ, IB], f32, tag="s0")
            for k in range(KC):
                nc.tensor.matmul(
                    sp[:], uwT[:, ds(k * JQ, JQ)], hT_all[:, ds(k * JX + b * IB, IB)],
                    start=(k == 0), stop=(k == KC - 1),
                )
            # ET = exp(sT + uwu[j]); uwu is the per-partition (j) ACT bias
            exp_inst = nc.scalar.activation(
                ET[:, ds(b * IB, IB)], sp[:], EXP, bias=uwu[:]
            )
            if b == 0:
                for ho in hout_late:
                    add_dep_helper(ho.ins, exp_inst.ins, sync=True,
                                   reason="delay h passthrough into DMA lull")
            # re-transpose ET (4 tiles into one bank); batched 3D reduces
            et = ps.tile([P, TPB * P], f32r, tag="tp")
            for q in range(TPB):
                t = b * TPB + q
                nc.tensor.transpose(
                    et[:, ds(q * P, P)], ET[:, ds(t * P, P)], ident[:]
                )
            et3 = et[:].rearrange("p (q x) -> p q x", q=TPB)
            nc.vector.reduce_max(m_exp[:, ds(b * TPB, TPB)], et3, axis=AX)
            zsum = stage.tile([P, TPB], f32, tag="zs")
            nc.vector.reduce_sum(zsum[:], et3, axis=AX)
            nc.vector.reciprocal(z_rec[:, ds(b * TPB, TPB)], zsum[:])

        def hap_block(b):
            # q2c accumulation (single PSUM group spanning both blocks)
            for q in range(TPB):
                t = b * TPB + q
                nc.tensor.matmul(
                    hap[:], m_exp[:, ds(t, 1)], h_all[:, ds(t * D, D)],
                    start=(b == 0 and q == 0), stop=(b == NB - 1 and q == TPB - 1),
                    skip_group_check=True,
                )

        def c2q_tile(t):
            up = ps.tile([P, D], f32, tag="ua")
            nc.tensor.matmul(
                up[:], ET[:, ds(t * P, P)], u_sb[:], start=True, stop=True
            )
            stg = stage.tile([P, 2 * D], f32, tag="stg")
            nc.scalar.mul(stg[:, ds(0, D)], up[:], z_rec[:, ds(t, 1)])
            nc.vector.scalar_tensor_tensor(
                stg[:, ds(D, D)], up[:], z_rec[:, ds(t, 1)], h_f[:, ds(t * D, D)],
                op0=mybir.AluOpType.mult, op1=mybir.AluOpType.mult,
            )
            nc.sync.dma_start(out_d[ds(t * P, P), ds(D, 2 * D)], stg[:])

        block_scores(0)
        for t in range(TPB):
            c2q_tile(t)
        hap_block(0)
        for t in range(TPB, NT):
            transpose_tile(t)
        block_scores(1)
        hap_block(1)

        # q2c chain right after block-1's reduce: bc becomes ready while
        # the block-0 stg writes still stream, so o4 spreads instead of
        # cramming at the end.
        mrow = consts.tile([P, 1], f32)
        nc.vector.reduce_sum(mrow[:], m_exp[:].bitcast(f32), axis=AX)
        zqp = ps.tile([1, 1], f32, tag="acc", bufs=1)
        nc.tensor.matmul(zqp[:], mrow[:], ones_col[:], start=True, stop=True)
        rzq = consts.tile([1, 1], f32)
        nc.vector.reciprocal(rzq[:], zqp[:])
        ha_sum = consts.tile([1, D], f32)
        nc.vector.tensor_copy(ha_sum[:], hap[:])
        ha_row = consts.tile([1, D], f32r)
        nc.scalar.mul(ha_row[:], ha_sum[:], rzq[:])
        bc = ps.tile([P, D], f32, tag="acc", bufs=1)
        nc.tensor.matmul(bc[:], ones_row[:], ha_row[:], start=True, stop=True)
        bc_sb = consts.tile([P, D], f32)
        nc.scalar.copy(bc_sb[:], bc[:])

        # block-1 c2q + o4 (col 3) for all tiles; o4 muls split DVE/GpSimd
        def o4_tile(t):
            o4 = stage.tile([P, D], f32, tag="o4")
            if t % 2 == 0:
                nc.vector.tensor_mul(o4[:], h_f[:, ds(t * D, D)], bc[:])
            else:
                nc.gpsimd.tensor_mul(o4[:], h_f[:, ds(t * D, D)], bc_sb[:])
            nc.sync.dma_start(out_d[ds(t * P, P), ds(3 * D, D)], o4[:])

        for q in range(TPB):
            c2q_tile(TPB + q)
            o4_tile(2 * q)
            o4_tile(2 * q + 1)

    nc.compile()
    return nc


def _get_nc():
    if "nc" not in _CACHE:
        _CACHE["nc"] = _build_program()
    return _CACHE["nc"]


def _ensure_axon_hooks_stub():
    # concourse imports antenv.axon_hooks when tracing is requested via env;
    # provide a no-op stub if the image lacks it so runs degrade gracefully.
    import sys
    import types

    try:
        import antenv.axon_hooks  # noqa: F401
    except ImportError:
        mod = types.ModuleType("antenv.axon_hooks")
        _hook = [None]
        mod.set_axon_ntff_profile_hook = lambda hook: _hook.__setitem__(0, hook)
        mod.get_axon_ntff_profile_hook = lambda: _hook[0]
        sys.modules["antenv.axon_hooks"] = mod


def kernel(h, u, alpha_w, alpha_b=None, **_unused):
    _ensure_axon_hooks_stub()
    from concourse.bass_utils import run_bass_kernel_spmd

    h = np.ascontiguousarray(np.asarray(h, dtype=np.float32)).reshape(N_B, JX, D)
    u = np.ascontiguousarray(np.asarray(u, dtype=np.float32)).reshape(N_B, JQ, D)
    alpha_w = np.ascontiguousarray(np.asarray(alpha_w, dtype=np.float32)).reshape(3 * D)

    nc = _get_nc()
    in_maps = [
        {"h": h[n], "u": u[n], "alpha_w": alpha_w} for n in range(N_B)
    ]
    res = run_bass_kernel_spmd(nc, in_maps, core_ids=list(range(N_B)))
    out = np.stack([res.results[n]["out"] for n in range(N_B)], axis=0)
    return out.reshape(N_B, M_B, JX, 4 * D)



# revision 5
# speedup vs baseline: 1.0896x; 1.0896x over previous
"""Trainium2 Bass kernel for BiDAF-style bidirectional attention.

Reference computation (per batch element n; M=1 folded away):
    s[i,j]  = h[i].w_h + u[j].w_u + (h[i]*u[j]).w_hu + b      [JX, JQ]
    a_u     = softmax_j(s);     u_a[i] = sum_j a_u[i,j] u[j]   (c2q)
    a_h     = softmax_i(max_j s);  h_a = sum_i a_h[i] h[i]     (q2c)
    out     = concat(h, u_a, h*u_a, h*h_a)                     [JX, 4D]

Sharding: data-parallel over batch N=8, one NeuronCore per batch element.
alpha_b drops out entirely (both softmaxes are shift-invariant).

v2 strategy (vs the 51us f32 baseline):
  - bf16 stores: the output tensor is bf16 (host casts back to f32).  Store
    traffic halves (8MB -> 4MB) and descriptor rows shrink; the 2e-2 rel-err
    gate leaves ~5x margin over bf16 rounding (~4e-3).
  - bf16 matmul operands (hT, ET, u, uw', m_exp): PE moving cost drops from
    1.5 cyc/col (f32r) to 1 cyc/col, evictions cast-on-copy to bf16 so the
    PSUM->SBUF eviction bandwidth halves too.  All accumulation stays f32
    (PSUM) so only operand rounding is lost.
  - Few big DMAs: h loads as 4 pair-tile transfers; output writes as 6
    merged 4-tile transfers (pass a/b, stg 0/1, o4 a/b) split across the
    two HWDGE queues (sync=loads+pass+o4, scalar=stg).  Each dma_start
    costs ~0.6us sequencer issue + ~0.9us completion, so 12 total DMA
    instructions instead of 26.
  - Flat PE spine: all 8 h-tile transposes run up front, block-1 scores
    issue right after block-0's ET retranspose, so EXP-1 lands ~10us
    earlier than the baseline's load-balanced two-phase schedule.
  - q2c tail: hap accumulation is prioritized on PE over the c2q matmuls;
    1/sum(m_exp) is partition-broadcast via a tiny K=1 matmul (rzqb) so
    o4 = (h * rzqb) * bc is a single scalar_tensor_tensor per tile with no
    serial scalar-multiply hop; the ha_row eviction is a fused cast.
"""

import numpy as np

N_B, M_B, JX, JQ, D = 8, 1, 1024, 128, 512
P = 128
NT = JX // P   # 8 i-tiles
KC = D // P    # 4 d-chunks
IB = 512       # i-block width for score matmuls
NB = JX // IB  # 2 blocks
TPB = NT // NB  # tiles per block

_CACHE = {}


def _build_program():
    from contextlib import ExitStack

    import concourse.bass as bass
    import concourse.tile as tile
    from concourse import bacc, mybir
    from concourse.masks import make_identity

    f32 = mybir.dt.float32
    f32r = mybir.dt.float32r
    bf16 = mybir.dt.bfloat16
    EXP = mybir.ActivationFunctionType.Exp
    AX = mybir.AxisListType.X
    MUL = mybir.AluOpType.mult
    ds = bass.ds

    nc = bacc.Bacc("TRN2", target_bir_lowering=False, debug=False, num_devices=8)
    h_d = nc.dram_tensor("h", [JX, D], f32, kind="ExternalInput").ap()
    u_d = nc.dram_tensor("u", [JQ, D], f32, kind="ExternalInput").ap()
    aw_d = nc.dram_tensor("alpha_w", [3 * D], f32, kind="ExternalInput").ap()
    out_d = nc.dram_tensor("out", [JX, 4 * D], bf16, kind="ExternalOutput").ap()

    with tile.TileContext(nc) as tc, ExitStack() as ctx:
        consts = ctx.enter_context(tc.tile_pool(name="consts", bufs=1))
        stage = ctx.enter_context(tc.tile_pool(name="stage", bufs=4))
        # PSUM budget (8 banks): tp=2, s0=2, ua=2, acc=1, hap=1
        ps = ctx.enter_context(tc.tile_pool(name="ps", bufs=2, space="PSUM"))

        # ---- PE warmup: f32r matmuls depending only on DVE ops, emitted
        # first so the HAM clock-gate opens (1.2 -> 2.4 GHz) while the h
        # DMAs stream in.
        warm_f = consts.tile([P, D], f32)
        nc.vector.memset(warm_f[:], 0.25)
        warm = consts.tile([P, D], f32r)
        nc.vector.tensor_copy(warm[:], warm_f[:])
        wp = ps.tile([P, D], f32, tag="acc", bufs=1)
        for w in range(4):
            nc.tensor.matmul(
                wp[:], warm[:, ds(0, P)], warm[:], start=True, stop=True,
            )

        # ---- constants ----
        ident_f = consts.tile([P, P], f32)
        make_identity(nc, ident_f[:])
        ident = consts.tile([P, P], f32r)
        nc.vector.tensor_copy(ident[:], ident_f[:])
        ident16 = consts.tile([P, P], bf16)
        nc.vector.tensor_copy(ident16[:], ident_f[:])
        ones_row_f = consts.tile([1, P], f32)
        nc.vector.memset(ones_row_f[:], 1.0)
        ones_row = consts.tile([1, P], f32r)
        nc.scalar.copy(ones_row[:], ones_row_f[:])
        ones_row16 = consts.tile([1, P], bf16)
        nc.scalar.copy(ones_row16[:], ones_row_f[:])
        ones_col = consts.tile([P, 1], f32)
        nc.vector.memset(ones_col[:], 1.0)

        # ---- loads (sync queue): u + aw first (they feed the uw' prep
        # chain), then h as 4 pair-tile DMAs.
        u_sb = consts.tile([JQ, D], f32r)
        nc.sync.dma_start(u_sb[:], u_d[:].bitcast(f32r))
        u_f = u_sb[:].bitcast(f32)
        aw_sb = consts.tile([1, 3 * D], f32r)
        nc.sync.dma_start(aw_sb[:], aw_d.rearrange("(o d) -> o d", o=1).bitcast(f32r))

        h_all = consts.tile([P, NT * D], f32r)    # tile t: h[t*128+p, d]
        h_f = h_all[:].bitcast(f32)
        for q in range(NT // 2):
            nc.sync.dma_start(
                h_all[:, ds(2 * q * D, 2 * D)].rearrange("p (t d) -> p t d", t=2),
                h_d[ds(2 * q * P, 2 * P), :]
                .rearrange("(t p) d -> p t d", p=P).bitcast(f32r),
            )

        # ---- bf16 companions (GpSimd stream) ----
        u16 = consts.tile([JQ, D], bf16)
        nc.gpsimd.tensor_copy(u16[:], u_f)

        # alpha_w partition-broadcast on-chip: K=1 matmuls into s0 PSUM,
        # read directly by the DVE/GpSimd.  whu first (gates uw').
        def wcast(c):
            wt = ps.tile([P, D], f32, tag="s0")
            nc.tensor.matmul(
                wt[:], ones_row[:], aw_sb[:, ds(c * D, D)], start=True, stop=True
            )
            return wt

        whu_p = wcast(2)
        wh_p = wcast(0)

        # uw[j,d] = u[j,d]*w_hu[d] + w_h[d];  uwu[j] = sum_d u[j,d]*w_u[d]
        uw = consts.tile([JQ, D], f32r)
        uw0 = consts.tile([JQ, D], f32)
        nc.vector.tensor_mul(uw0[:], u_f, whu_p[:])
        nc.vector.tensor_add(uw[:], uw0[:], wh_p[:])
        wu_p = wcast(1)
        uwtmp = consts.tile([JQ, D], f32)
        uwu = consts.tile([JQ, 1], f32)
        nc.vector.scalar_tensor_tensor(
            uwtmp[:], u_f, 1.0, wu_p[:],
            op0=MUL, op1=MUL, accum_out=uwu[:],
        )

        # h16 casts (GpSimd): feed the passthrough writes, hap moving
        # operand, stg col2 and o4 muls.
        h16 = consts.tile([P, NT * D], bf16)
        for t in range(NT):
            nc.gpsimd.tensor_copy(h16[:, ds(t * D, D)], h_f[:, ds(t * D, D)])

        # ---- hT via PE transposes (f32r moving ident), cast-evict to bf16.
        hT16 = consts.tile([P, KC * JX], bf16)    # chunk k: hT[k*128+p, i]
        hT3 = hT16[:].rearrange("p (k x) -> p k x", k=KC)

        def transpose_tile(t):
            pt = ps.tile([P, KC * P], f32r, tag="tp")
            for k in range(KC):
                nc.tensor.transpose(
                    pt[:, ds(k * P, P)], h_all[:, ds(t * D + k * P, P)], ident[:]
                )
            ev = nc.scalar.copy if t % 4 == 0 else nc.vector.tensor_copy
            ev(hT3[:, :, ds(t * P, P)],
               pt[:].bitcast(f32).rearrange("p (k x) -> p k x", k=KC))

        transpose_tile(0)
        transpose_tile(1)

        # uwT16[d_chunk][j]: 4 transposes into one PSUM bank, one cast-evict
        uwT16 = consts.tile([P, KC * JQ], bf16)
        ptw = ps.tile([P, KC * P], f32r, tag="tp")
        for k in range(KC):
            nc.tensor.transpose(ptw[:, ds(k * P, P)], uw[:, ds(k * P, P)], ident[:])
        nc.scalar.copy(uwT16[:], ptw[:].bitcast(f32))

        for t in range(2, NT):
            transpose_tile(t)

        # ---- passthrough writes (col 0 = h, bf16), 4 tiles per DMA on sync
        def pass_write(t0):
            nc.sync.dma_start(
                out_d[ds(t0 * P, TPB * P), ds(0, D)]
                .rearrange("(t p) c -> p t c", p=P),
                h16[:, ds(t0 * D, TPB * D)].rearrange("p (t c) -> p t c", t=TPB),
            )

        # ---- scores (transposed layout): sT[j,i] over a 512-wide i-block
        ET16 = consts.tile([JQ, JX], bf16)        # exp(sT + uwu[j]) = exp(s - b)
        m16 = consts.tile([P, NT], bf16)          # per i-tile: max_j ET
        z_rec = consts.tile([P, NT], f32)         # per i-tile: 1/sum_j ET

        def block_scores(b):
            sp = ps.tile([JQ, IB], f32, tag="s0")
            for k in range(KC):
                nc.tensor.matmul(
                    sp[:], uwT16[:, ds(k * JQ, JQ)], hT3[:, k, ds(b * IB, IB)],
                    start=(k == 0), stop=(k == KC - 1),
                )
            # ET = exp(sT + uwu[j]); uwu is the per-partition (j) ACT bias
            nc.scalar.activation(ET16[:, ds(b * IB, IB)], sp[:], EXP, bias=uwu[:])

        def block_reduce(b):
            # re-transpose ET (4 tiles into one bank, bf16); batched reduces
            et = ps.tile([P, TPB * P], bf16, tag="tp")
            for q in range(TPB):
                t = b * TPB + q
                nc.tensor.transpose(
                    et[:, ds(q * P, P)], ET16[:, ds(t * P, P)], ident16[:]
                )
            et3 = et[:].rearrange("p (q x) -> p q x", q=TPB)
            nc.vector.reduce_max(m16[:, ds(b * TPB, TPB)], et3, axis=AX)
            zsum = stage.tile([P, TPB], f32, tag="zs")
            nc.vector.reduce_sum(zsum[:], et3, axis=AX)
            nc.vector.reciprocal(z_rec[:, ds(b * TPB, TPB)], zsum[:])

        hap = ps.tile([1, D], f32, tag="hap", bufs=1)

        def hap_block(b):
            # q2c accumulation (single PSUM group spanning both blocks)
            for q in range(TPB):
                t = b * TPB + q
                nc.tensor.matmul(
                    hap[:], m16[:, ds(t, 1)], h16[:, ds(t * D, D)],
                    start=(b == 0 and q == 0), stop=(b == NB - 1 and q == TPB - 1),
                    skip_group_check=True,
                )

        # c2q per tile: up = a_u-unnormalized @ u; stg cols 1-2
        stg16 = consts.tile([P, NT * 2 * D], bf16)

        def c2q_tile(t):
            up = ps.tile([P, D], f32, tag="ua")
            nc.tensor.matmul(
                up[:], ET16[:, ds(t * P, P)], u16[:], start=True, stop=True
            )
            zr = z_rec[:, ds(t, 1)]
            # col1 = up * zr reads PSUM (DVE/ACT only: GpSimd has no PSUM
            # port); col2 = col1 * h16 is then all-SBUF bf16 so GpSimd can
            # carry half of it.
            c1 = stg16[:, ds(t * 2 * D, D)]
            if t % 2 == 0:
                nc.scalar.mul(c1, up[:], zr)
            else:
                nc.vector.tensor_scalar_mul(c1, up[:], zr)
            eng = nc.vector if t % 2 == 0 else nc.gpsimd
            eng.tensor_mul(stg16[:, ds(t * 2 * D + D, D)], c1, h16[:, ds(t * D, D)])

        def stg_write(t0):
            nc.scalar.dma_start(
                out_d[ds(t0 * P, TPB * P), ds(D, 2 * D)]
                .rearrange("(t p) c -> p t c", p=P),
                stg16[:, ds(t0 * 2 * D, TPB * 2 * D)]
                .rearrange("p (t c) -> p t c", t=TPB),
            )

        # ---- spine ----
        pass_write(0)
        block_scores(0)
        block_reduce(0)
        block_scores(1)
        pass_write(TPB)
        # early c2q for tiles 0-2 runs in the EXP-1 / reduce-1 shadow
        c2q_tile(0)
        c2q_tile(1)
        c2q_tile(2)
        block_reduce(1)
        c2q_tile(3)
        stg_write(0)
        hap_block(0)
        hap_block(1)

        # q2c normalization chain: rzq = 1/sum_i m_exp[i] folds into the
        # ha_row eviction as an ACT scale, so bc is the NORMALIZED h_a
        # broadcast and o4 is a plain tensor_mul per tile.
        mrow = consts.tile([P, 1], f32)
        nc.vector.reduce_sum(mrow[:], m16[:], axis=AX)
        zqp = ps.tile([1, 1], f32, tag="acc", bufs=1)
        nc.tensor.matmul(zqp[:], mrow[:], ones_col[:], start=True, stop=True)
        rzq = consts.tile([1, 1], f32)
        nc.vector.reciprocal(rzq[:], zqp[:])
        ha_row = consts.tile([1, D], bf16)
        nc.scalar.mul(ha_row[:], hap[:], rzq[:])
        bc = ps.tile([P, D], f32, tag="acc", bufs=1)
        nc.tensor.matmul(bc[:], ones_row16[:], ha_row[:], start=True, stop=True)
        bc_sb = consts.tile([P, D], bf16)
        nc.vector.tensor_copy(bc_sb[:], bc[:])

        o4_16 = consts.tile([P, NT * D], bf16)

        def o4_tile(t):
            if t % 2 == 0:
                nc.vector.tensor_mul(o4_16[:, ds(t * D, D)], h16[:, ds(t * D, D)], bc[:])
            else:
                nc.gpsimd.tensor_mul(
                    o4_16[:, ds(t * D, D)], h16[:, ds(t * D, D)], bc_sb[:]
                )

        def o4_write(t0):
            nc.sync.dma_start(
                out_d[ds(t0 * P, TPB * P), ds(3 * D, D)]
                .rearrange("(t p) c -> p t c", p=P),
                o4_16[:, ds(t0 * D, TPB * D)].rearrange("p (t c) -> p t c", t=TPB),
            )

        for t in range(TPB):
            o4_tile(t)
        c2q_tile(TPB)
        c2q_tile(TPB + 1)
        o4_write(0)
        for t in range(TPB, NT):
            o4_tile(t)
        c2q_tile(TPB + 2)
        c2q_tile(TPB + 3)
        o4_write(TPB)
        stg_write(TPB)

    nc.compile()
    return nc


def _get_nc():
    if "nc" not in _CACHE:
        _CACHE["nc"] = _build_program()
    return _CACHE["nc"]


def _ensure_axon_hooks_stub():
    # concourse imports antenv.axon_hooks when tracing is requested via env;
    # provide a no-op stub if the image lacks it so runs degrade gracefully.
    import sys
    import types

    try:
        import antenv.axon_hooks  # noqa: F401
    except ImportError:
        mod = types.ModuleType("antenv.axon_hooks")
        _hook = [None]
        mod.set_axon_ntff_profile_hook = lambda hook: _hook.__setitem__(0, hook)
        mod.get_axon_ntff_profile_hook = lambda: _hook[0]
        sys.modules["antenv.axon_hooks"] = mod


def _postprocess(res):
    out = np.stack(
        [np.asarray(res.results[n]["out"]).astype(np.float32) for n in range(N_B)],
        axis=0,
    )
    return out.reshape(N_B, M_B, JX, 4 * D)


def kernel(h, u, alpha_w, alpha_b=None, **_unused):
    _ensure_axon_hooks_stub()
    from concourse.bass_utils import run_bass_kernel_spmd

    h = np.ascontiguousarray(np.asarray(h, dtype=np.float32)).reshape(N_B, JX, D)
    u = np.ascontiguousarray(np.asarray(u, dtype=np.float32)).reshape(N_B, JQ, D)
    alpha_w = np.ascontiguousarray(np.asarray(alpha_w, dtype=np.float32)).reshape(3 * D)

    nc = _get_nc()
    in_maps = [
        {"h": h[n], "u": u[n], "alpha_w": alpha_w} for n in range(N_B)
    ]
    res = run_bass_kernel_spmd(nc, in_maps, core_ids=list(range(N_B)))
    return _postprocess(res)
